# revision 27
# baseline (speedup 1.0000x reference)
"""Trainium2 kernel for nn_Attend_13537736916998 (sparse_attention).

Mathematical reduction of the reference:
  - sim <= 0 everywhere, so the selective-attention gate relu(sim[:, 0]) is
    identically zero -> the gate/cumsum branch is a numerical no-op.
  - attn = hard + soft - stop_gradient(soft) evaluates elementwise to the
    one-hot `hard` (+ O(2^-24)).  Hence
    out[b,h,i,:] = v[b,h, argmax_{j<=i} (q_i.k_j - 0.5||k_j||^2), :].

Score matmul: exact-enough 2-pass fp16 limb decomposition (1 cyc/row/pass
on the PE instead of fp32's 4):
  pass1: [qhi; qlo]^T @ [khi; khi]   = (qhi+qlo).khi
  pass2: [qhi; 1; 1]^T @ [klo; b1; b2] = qhi.klo + b      (b = -0.5||k||^2)
plus a third tiny matmul on the diagonal 128-block adding -60000*[j > i]
(tri^T @ (-60000*I)), which implements the causal mask inside PSUM and keeps
the mask off the vector/gpsimd critical path.
Verified: 0/32768 output rows differ from the fp32 reference argmax.

Per 128-row tile: scalar engine copies PSUM chunks to SBUF, vector max8 +
find_index8 give the causal argmax, gpsimd indirect DMA gathers the winning
v rows from HBM.  fp16 limb subtractions run on gpsimd; dtype-dup rows are
materialized by SBUF-to-SBUF DMA to keep the vector engine on scans only.

Output is emitted in gather layout [2, 128, 16, 64] (partition-major) and
re-ordered on the host during unsharding.
"""

import numpy as np
from contextlib import ExitStack

import concourse.bass as bass
import concourse.bacc as bacc
import concourse.tile as tile
from concourse import mybir
import concourse.bass_utils as bass_utils

B, H, N, D = 2, 8, 2048, 64
P = 128
NT = N // P            # 16 row tiles per (b,h) pair
T = 2                  # (b,h) pairs per core
NCORES = 8
F32 = mybir.dt.float32
F16 = mybir.dt.float16
U32 = mybir.dt.uint32
MASKVAL = -60000.0     # fp16-representable; dwarfs any valid score


def kernel_body(tc, qa, kt, v, out):
    nc = tc.nc
    with ExitStack() as ctx:
        consts = ctx.enter_context(tc.tile_pool(name="consts", bufs=1))
        io = ctx.enter_context(tc.tile_pool(name="io", bufs=2))
        work = ctx.enter_context(tc.tile_pool(name="work", bufs=6))
        outp = ctx.enter_context(tc.tile_pool(name="outp", bufs=2))
        small = ctx.enter_context(tc.tile_pool(name="small", bufs=6))
        ps_pool = ctx.enter_context(tc.tile_pool(name="ps", bufs=7, space="PSUM"))
        psk_pool = ctx.enter_context(tc.tile_pool(name="psk", bufs=1, space="PSUM"))

        ones_col = consts.tile([D, 1], F32)
        nc.vector.memset(ones_col, 1.0)
        ones2 = consts.tile([1, 2, N], F16)     # staged [1;1] rows for qho
        nc.vector.memset(ones2, 1.0)
        # causal-mask matmul constants: tri[d,i] = 1[d > i]; negI = MASKVAL*I
        triA = consts.tile([P, P], F16)
        nc.vector.memset(triA, 1.0)
        nc.gpsimd.affine_select(out=triA, in_=triA, pattern=[[-1, P]], base=-1,
                                channel_multiplier=1,
                                compare_op=mybir.AluOpType.is_ge, fill=0.0)
        negI = consts.tile([P, P], F16)
        nc.vector.memset(negI, MASKVAL)
        nc.gpsimd.affine_select(out=negI, in_=negI, pattern=[[-1, P]], base=0,
                                channel_multiplier=1,
                                compare_op=mybir.AluOpType.is_equal, fill=0.0)

        def emit_prep(t):
            # ---- q-side prep: fp32 load + fp16 limb split, high cols first
            qa_t = io.tile([D + 1, N], F32, tag="qa")
            qhl = io.tile([2 * D, N], F16, tag="qhl")      # [qhi; qlo]
            qho = io.tile([D + 2, N], F16, tag="qho")      # [qhi; 1; 1]
            nc.sync.dma_start(out=qho[D:D + 2, :], in_=ones2[:, :, :])
            for c in reversed(range(N // 512)):
                cs = slice(c * 512, (c + 1) * 512)
                nc.sync.dma_start(out=qa_t[:, cs], in_=qa[t][:, cs])
                nc.scalar.copy(qhl[0:D, cs], qa_t[0:D, cs])            # qhi
                nc.gpsimd.tensor_sub(qhl[D:2 * D, cs], qa_t[0:D, cs],
                                     qhl[0:D, cs])                     # qlo
                nc.sync.dma_start(out=qho[0:D, cs], in_=qhl[0:D, cs])  # dup

            # ---- k-side prep: fp16 limbs + fp32 ksq bias (split to fp16)
            kt_t = io.tile([D, N], F32, tag="kt")
            sq = io.tile([D, N], F32, tag="sq")
            khh = io.tile([2 * D, N], F16, tag="khh")      # [khi; khi]
            klb = io.tile([D + 2, N], F16, tag="klb")      # [klo; b1; b2]
            b32 = io.tile([1, N], F32, tag="b32")
            bb = io.tile([1, 2, N], F16, tag="bb")
            for c in range(N // 512):
                cs = slice(c * 512, (c + 1) * 512)
                nc.sync.dma_start(out=kt_t[:, cs], in_=kt[t][:, cs])
                nc.scalar.copy(khh[0:D, cs], kt_t[:, cs])              # khi
                nc.gpsimd.tensor_sub(klb[0:D, cs], kt_t[:, cs],
                                     khh[0:D, cs])                     # klo
                nc.sync.dma_start(out=khh[D:2 * D, cs], in_=khh[0:D, cs])
                nc.scalar.square(sq[:, cs], kt_t[:, cs])
                pk = psk_pool.tile([1, 512], F32, tag="pk")
                nc.tensor.matmul(pk, lhsT=ones_col, rhs=sq[:, cs],
                                 start=True, stop=True)
                nc.scalar.mul(b32[:, cs], pk, -0.5)
                nc.scalar.copy(bb[:, 0, cs], b32[:, cs])               # b1
                nc.gpsimd.tensor_sub(bb[:, 1, cs], b32[:, cs],
                                     bb[:, 0, cs])                     # b2
                nc.sync.dma_start(out=klb[D:D + 2, cs], in_=bb[:, :, cs])
            idxs = outp.tile([P, NT, 8], U32, tag="idxs")
            vout = outp.tile([P, NT, D], F32, tag="vout")
            return qhl, qho, khh, klb, sq, idxs, vout

        def emit_mm1(t, st, m):
            # all pass-1 chunks share one qhl weight load, then the mask,
            # then all pass-2 chunks share one qho load: 3 LDWEIGHTS per
            # tile instead of 2 per chunk, and a denser PE matmul stream.
            qhl, qho, khh, klb, sq, idxs, vout = st
            W = (m + 1) * P
            ms = slice(m * P, (m + 1) * P)
            S = work.tile([P, N], F32, tag="S")
            nchunks = (W + 511) // 512
            bounds = []
            pss = []
            for c in range(nchunks):
                lo = c * 512
                hi = min(W, lo + 512)
                bounds.append((lo, hi))
                ps = ps_pool.tile([P, 512], F32, tag="ps")
                pss.append(ps)
                nc.tensor.matmul(ps[:, : hi - lo], lhsT=qhl[:, ms],
                                 rhs=khh[:, lo:hi], start=True, stop=False)
            # causal mask on the diagonal 128 cols via the PE:
            # adds MASKVAL*[j > i] mid-accumulation-group.
            dlo = (nchunks - 1) * 512
            nc.tensor.matmul(pss[-1][:, W - P - dlo:W - dlo], lhsT=triA,
                             rhs=negI, start=False, stop=False)
            return S, bounds, pss

        def emit_rest(t, st, m, h):
            qhl, qho, khh, klb, sq, idxs, vout = st
            S, bounds, pss = h
            W = (m + 1) * P
            ms = slice(m * P, (m + 1) * P)
            for c, (lo, hi) in enumerate(bounds):
                ps = pss[c]
                nc.tensor.matmul(ps[:, : hi - lo], lhsT=qho[:, ms],
                                 rhs=klb[:, lo:hi], start=False, stop=True)
                nc.scalar.copy(S[:, lo:hi], ps[:, : hi - lo])
            mx = small.tile([P, 8], F32, tag="mx")
            nc.vector.max(mx, S[:, 0:W])
            nc.vector.max_index(idxs[:, m, :], mx, S[:, 0:W])
            # gather the 128 winning v rows for this row tile.
            # NB: one offset column per indirect DMA — multi-column offset
            # tables mis-generate descriptors on HW.
            nc.gpsimd.indirect_dma_start(
                out=vout[:, m, :],
                out_offset=None,
                in_=v,
                in_offset=bass.IndirectOffsetOnAxis(ap=idxs[:, m, 0:1], axis=1),
                element_offset=t * N * D,
            )

        # big/small interleave: PE stays fed with large tiles while the
        # vector engine's backlog drains on small ones; each pair ends on
        # the cheapest tiles so the end-of-kernel tail is short.
        # pair 1 ends on the two cheapest tiles (m=8 then m=2, with m=7
        # moved to mid-stream) so the final scan->gather->out-DMA tail is
        # as short as possible; pair 0's tail overlaps pair-1 prep anyway.
        orders = [
            [15, 0, 14, 1, 13, 2, 12, 3, 11, 4, 10, 5, 9, 6, 8, 7],
            [15, 0, 14, 1, 13, 7, 12, 3, 11, 4, 10, 5, 9, 6, 8, 2],
        ]
        for t in range(T):
            order = orders[t]
            st = emit_prep(t)
            for m in order:
                h = emit_mm1(t, st, m)
                emit_rest(t, st, m, h)
            vout = st[6]
        # two half-writes: the m=15..8 gathers finish long before the rest
            nc.sync.dma_start(out=out[t][:, 8:NT, :], in_=vout[:, 8:NT, :])
            nc.sync.dma_start(out=out[t][:, 0:8, :], in_=vout[:, 0:8, :])


_NC_CACHE = None


def build_nc():
    global _NC_CACHE
    if _NC_CACHE is not None:
        return _NC_CACHE
    nc = bacc.Bacc(
        "TRN2",
        target_bir_lowering=False,
        debug=False,
        enable_asserts=False,
        num_devices=NCORES,
    )
    qa = nc.dram_tensor("qa", [T, D + 1, N], F32, kind="ExternalInput").ap()
    kt = nc.dram_tensor("kt", [T, D, N], F32, kind="ExternalInput").ap()
    v = nc.dram_tensor("v", [T, N, D], F32, kind="ExternalInput").ap()
    out = nc.dram_tensor("out", [T, P, NT, D], F32, kind="ExternalOutput").ap()
    with tile.TileContext(nc) as tc:
        kernel_body(tc, qa, kt, v, out)
    nc.compile()
    _NC_CACHE = nc
    return nc


def make_in_maps(q, k, v):
    q = np.asarray(q, dtype=np.float32)
    k = np.asarray(k, dtype=np.float32)
    v = np.asarray(v, dtype=np.float32)
    assert q.shape == (B, H, N, D), q.shape
    in_maps = []
    for c in range(NCORES):
        qa_c = np.empty((T, D + 1, N), np.float32)
        kt_c = np.empty((T, D, N), np.float32)
        v_c = np.empty((T, N, D), np.float32)
        for t in range(T):
            gp = T * c + t
            b, h = divmod(gp, H)
            qa_c[t, :D] = q[b, h].T
            qa_c[t, D] = 1.0
            kt_c[t] = k[b, h].T
            v_c[t] = v[b, h]
        in_maps.append({"qa": qa_c, "kt": kt_c, "v": v_c})
    return in_maps


def unmarshal(results):
    out = np.empty((B, H, N, D), np.float32)
    for c in range(NCORES):
        o = np.asarray(results[c]["out"])  # [T, P, NT, D]
        for t in range(T):
            gp = T * c + t
            b, h = divmod(gp, H)
            out[b, h] = o[t].transpose(1, 0, 2).reshape(N, D)
    return out


def kernel(q, k, v):
    nc = build_nc()
    in_maps = make_in_maps(q, k, v)
    res = bass_utils.run_bass_kernel_spmd(nc, in_maps, core_ids=list(range(NCORES)))
    return unmarshal(res.results)


# revision 28
# speedup vs baseline: 1.1762x; 1.1762x over previous
"""Trainium2 kernel for nn_Attend_13537736916998 (sparse_attention).

Mathematical reduction of the reference:
  - sim <= 0 everywhere, so the selective-attention gate relu(sim[:, 0]) is
    identically zero -> the gate/cumsum branch is a numerical no-op.
  - attn = hard + soft - stop_gradient(soft) evaluates elementwise to the
    one-hot `hard` (+ O(2^-24)).  Hence
    out[b,h,i,:] = v[b,h, argmax_{j<=i} (q_i.k_j - 0.5||k_j||^2), :].

Score matmul: exact-enough 2-pass fp16 limb decomposition (1 cyc/row/pass
on the PE instead of fp32's 4):
  pass1: [qhi; qlo]^T @ [khi; khi]   = (qhi+qlo).khi
  pass2: [qhi; 1; 1]^T @ [klo; b1; b2] = qhi.klo + b      (b = -0.5||k||^2)
plus a third tiny matmul on the diagonal 128-block adding -60000*[j > i]
(tri^T @ (-60000*I)), which implements the causal mask inside PSUM and keeps
the mask off the vector/gpsimd critical path.
Verified: 0/32768 output rows differ from the fp32 reference argmax.

Per 128-row tile: scalar engine copies PSUM chunks to SBUF, vector max8 +
find_index8 give the causal argmax, gpsimd indirect DMA gathers the winning
v rows from HBM.  fp16 limb subtractions run on gpsimd; dtype-dup rows are
materialized by SBUF-to-SBUF DMA to keep the vector engine on scans only.

Output is emitted in gather layout [2, 128, 16, 64] (partition-major) and
re-ordered on the host during unsharding.
"""

import numpy as np
from contextlib import ExitStack

import concourse.bass as bass
import concourse.bacc as bacc
import concourse.tile as tile
from concourse import mybir
import concourse.bass_utils as bass_utils

B, H, N, D = 2, 8, 2048, 64
P = 128
NT = N // P            # 16 row tiles per (b,h) pair
T = 2                  # (b,h) pairs per core
NCORES = 8
F32 = mybir.dt.float32
F16 = mybir.dt.float16
U32 = mybir.dt.uint32
MASKVAL = -60000.0     # fp16-representable; dwarfs any valid score


def kernel_body(tc, qa, kt, v, out):
    nc = tc.nc
    with ExitStack() as ctx:
        consts = ctx.enter_context(tc.tile_pool(name="consts", bufs=1))
        io = ctx.enter_context(tc.tile_pool(name="io", bufs=2))
        work = ctx.enter_context(tc.tile_pool(name="work", bufs=5))
        outp = ctx.enter_context(tc.tile_pool(name="outp", bufs=2))
        small = ctx.enter_context(tc.tile_pool(name="small", bufs=6))
        ps_pool = ctx.enter_context(tc.tile_pool(name="ps", bufs=7, space="PSUM"))
        psk_pool = ctx.enter_context(tc.tile_pool(name="psk", bufs=1, space="PSUM"))

        ones_col = consts.tile([D, 1], F32)
        nc.vector.memset(ones_col, 1.0)
        ones2 = consts.tile([1, 2, N], F16)     # staged [1;1] rows for qho
        nc.vector.memset(ones2, 1.0)
        # causal-mask matmul constants: tri[d,i] = 1[d > i]; negI = MASKVAL*I
        triA = consts.tile([P, P], F16)
        nc.vector.memset(triA, 1.0)
        nc.gpsimd.affine_select(out=triA, in_=triA, pattern=[[-1, P]], base=-1,
                                channel_multiplier=1,
                                compare_op=mybir.AluOpType.is_ge, fill=0.0)
        negI = consts.tile([P, P], F16)
        nc.vector.memset(negI, MASKVAL)
        nc.gpsimd.affine_select(out=negI, in_=negI, pattern=[[-1, P]], base=0,
                                channel_multiplier=1,
                                compare_op=mybir.AluOpType.is_equal, fill=0.0)

        def emit_prep(t):
            # ---- q-side prep: fp32 load + fp16 limb split, high cols first
            qa_t = io.tile([D + 1, N], F32, tag="qa")
            qhl = io.tile([2 * D, N], F16, tag="qhl")      # [qhi; qlo]
            qho = io.tile([D + 2, N], F16, tag="qho")      # [qhi; 1; 1]
            nc.sync.dma_start(out=qho[D:D + 2, :], in_=ones2[:, :, :])
            for c in reversed(range(N // 512)):
                cs = slice(c * 512, (c + 1) * 512)
                nc.sync.dma_start(out=qa_t[:, cs], in_=qa[t][:, cs])
                nc.scalar.copy(qhl[0:D, cs], qa_t[0:D, cs])            # qhi
                nc.gpsimd.tensor_sub(qhl[D:2 * D, cs], qa_t[0:D, cs],
                                     qhl[0:D, cs])                     # qlo
                nc.sync.dma_start(out=qho[0:D, cs], in_=qhl[0:D, cs])  # dup

            # ---- k-side prep: fp16 limbs + fp32 ksq bias (split to fp16)
            kt_t = io.tile([D, N], F32, tag="kt")
            sq = io.tile([D, N], F32, tag="sq")
            khh = io.tile([2 * D, N], F16, tag="khh")      # [khi; khi]
            klb = io.tile([D + 2, N], F16, tag="klb")      # [klo; b1; b2]
            b32 = io.tile([1, N], F32, tag="b32")
            bb = io.tile([1, 2, N], F16, tag="bb")
            for c in range(N // 512):
                cs = slice(c * 512, (c + 1) * 512)
                nc.sync.dma_start(out=kt_t[:, cs], in_=kt[t][:, cs])
                nc.scalar.copy(khh[0:D, cs], kt_t[:, cs])              # khi
                nc.gpsimd.tensor_sub(klb[0:D, cs], kt_t[:, cs],
                                     khh[0:D, cs])                     # klo
                nc.sync.dma_start(out=khh[D:2 * D, cs], in_=khh[0:D, cs])
                nc.scalar.square(sq[:, cs], kt_t[:, cs])
                pk = psk_pool.tile([1, 512], F32, tag="pk")
                nc.tensor.matmul(pk, lhsT=ones_col, rhs=sq[:, cs],
                                 start=True, stop=True)
                nc.scalar.mul(b32[:, cs], pk, -0.5)
                nc.scalar.copy(bb[:, 0, cs], b32[:, cs])               # b1
                nc.gpsimd.tensor_sub(bb[:, 1, cs], b32[:, cs],
                                     bb[:, 0, cs])                     # b2
                nc.sync.dma_start(out=klb[D:D + 2, cs], in_=bb[:, :, cs])
            idxs = outp.tile([P, NT, 8], U32, tag="idxs")
            vout = outp.tile([P, NT, D], F32, tag="vout")
            return qhl, qho, khh, klb, sq, idxs, vout

        def emit_mm1(t, st, m):
            # all pass-1 chunks share one qhl weight load, then the mask,
            # then all pass-2 chunks share one qho load: 3 LDWEIGHTS per
            # tile instead of 2 per chunk, and a denser PE matmul stream.
            qhl, qho, khh, klb, sq, idxs, vout = st
            W = (m + 1) * P
            ms = slice(m * P, (m + 1) * P)
            S = work.tile([P, N], F32, tag="S")
            nchunks = (W + 511) // 512
            bounds = []
            pss = []
            for c in range(nchunks):
                lo = c * 512
                hi = min(W, lo + 512)
                bounds.append((lo, hi))
                ps = ps_pool.tile([P, 512], F32, tag="ps")
                pss.append(ps)
                nc.tensor.matmul(ps[:, : hi - lo], lhsT=qhl[:, ms],
                                 rhs=khh[:, lo:hi], start=True, stop=False)
            # causal mask on the diagonal 128 cols via the PE:
            # adds MASKVAL*[j > i] mid-accumulation-group.
            dlo = (nchunks - 1) * 512
            nc.tensor.matmul(pss[-1][:, W - P - dlo:W - dlo], lhsT=triA,
                             rhs=negI, start=False, stop=False)
            return S, bounds, pss

        def emit_rest(t, st, m, h):
            qhl, qho, khh, klb, sq, idxs, vout = st
            S, bounds, pss = h
            W = (m + 1) * P
            ms = slice(m * P, (m + 1) * P)
            for c, (lo, hi) in enumerate(bounds):
                ps = pss[c]
                nc.tensor.matmul(ps[:, : hi - lo], lhsT=qho[:, ms],
                                 rhs=klb[:, lo:hi], start=False, stop=True)
                nc.scalar.copy(S[:, lo:hi], ps[:, : hi - lo])
            mx = small.tile([P, 8], F32, tag="mx")
            nc.vector.max(mx, S[:, 0:W])
            nc.vector.max_index(idxs[:, m, :], mx, S[:, 0:W])
            # gather the 128 winning v rows for this row tile.
            # NB: one offset column per indirect DMA — multi-column offset
            # tables mis-generate descriptors on HW.
            nc.gpsimd.indirect_dma_start(
                out=vout[:, m, :],
                out_offset=None,
                in_=v,
                in_offset=bass.IndirectOffsetOnAxis(ap=idxs[:, m, 0:1], axis=1),
                element_offset=t * N * D,
            )

        # big/small interleave: PE stays fed with large tiles while the
        # vector engine's backlog drains on small ones; each pair ends on
        # the cheapest tiles so the end-of-kernel tail is short.
        order = [15, 0, 14, 1, 13, 2, 12, 3, 11, 4, 10, 5, 9, 6, 8, 7]
        for t in range(T):
            st = emit_prep(t)
            for m in order:
                h = emit_mm1(t, st, m)
                emit_rest(t, st, m, h)
            vout = st[6]
        # two half-writes: the m=15..8 gathers finish long before the rest
            nc.sync.dma_start(out=out[t][:, 8:NT, :], in_=vout[:, 8:NT, :])
            nc.sync.dma_start(out=out[t][:, 0:8, :], in_=vout[:, 0:8, :])


_NC_CACHE = None


def build_nc():
    global _NC_CACHE
    if _NC_CACHE is not None:
        return _NC_CACHE
    nc = bacc.Bacc(
        "TRN2",
        target_bir_lowering=False,
        debug=False,
        enable_asserts=False,
        num_devices=NCORES,
    )
    qa = nc.dram_tensor("qa", [T, D + 1, N], F32, kind="ExternalInput").ap()
    kt = nc.dram_tensor("kt", [T, D, N], F32, kind="ExternalInput").ap()
    v = nc.dram_tensor("v", [T, N, D], F32, kind="ExternalInput").ap()
    out = nc.dram_tensor("out", [T, P, NT, D], F32, kind="ExternalOutput").ap()
    with tile.TileContext(nc) as tc:
        kernel_body(tc, qa, kt, v, out)
    nc.compile()
    _NC_CACHE = nc
    return nc


def make_in_maps(q, k, v):
    q = np.asarray(q, dtype=np.float32)
    k = np.asarray(k, dtype=np.float32)
    v = np.asarray(v, dtype=np.float32)
    assert q.shape == (B, H, N, D), q.shape
    in_maps = []
    for c in range(NCORES):
        qa_c = np.empty((T, D + 1, N), np.float32)
        kt_c = np.empty((T, D, N), np.float32)
        v_c = np.empty((T, N, D), np.float32)
        for t in range(T):
            gp = T * c + t
            b, h = divmod(gp, H)
            qa_c[t, :D] = q[b, h].T
            qa_c[t, D] = 1.0
            kt_c[t] = k[b, h].T
            v_c[t] = v[b, h]
        in_maps.append({"qa": qa_c, "kt": kt_c, "v": v_c})
    return in_maps


def unmarshal(results):
    out = np.empty((B, H, N, D), np.float32)
    for c in range(NCORES):
        o = np.asarray(results[c]["out"])  # [T, P, NT, D]
        for t in range(T):
            gp = T * c + t
            b, h = divmod(gp, H)
            out[b, h] = o[t].transpose(1, 0, 2).reshape(N, D)
    return out


def kernel(q, k, v):
    nc = build_nc()
    in_maps = make_in_maps(q, k, v)
    res = bass_utils.run_bass_kernel_spmd(nc, in_maps, core_ids=list(range(NCORES)))
    return unmarshal(res.results)
